# revision 41
# baseline (speedup 1.0000x reference)
"""CMXBlock (dense transformer block) Trainium2 Bass kernel.

Sharding: data-parallel over batch B=8 across the 8 NeuronCores — one image
per core, all weights replicated, no collectives.

Per-core computation (C=256 channels on partitions, HW=1024 positions free):
  x1 <- x1 + proj(softmax((q_w@bn1(x1))^T (k_w@bn1(x2)) * temp) @ (v_w@bn1(x2))^T)
  x1 <- x1 + fc2(gelu(dwconv3x3(fc1(bn2(x1)))))

The kernel is ACT(exp)-bound in attention and PE-bound in the MLP; the
HAM clock gate never warms during attention (small-K scores / M=64 AV do
not register as PE activity), so the attention pipeline is organized to
hide the 1.2 GHz PE entirely under the exp stream.

Key structure:
 - BatchNorms/temp folded into conv weights host-side; per-partition biases
   applied at PSUM eviction.  q/k in bf16, scores K=32 f32 PSUM.
 - Attention processes head pairs (h, h+2) with members interleaved per
   m-tile: scores run row-concurrent on PE row groups (hb, hb+64), the exp
   of each [128,1024] score tile goes ACT->bf16 SBUF directly, and the AV
   matmuls of the two members run column-concurrent (M=64 stationaries
   [v | ones] at PE columns 0/64) accumulating U and the softmax
   denominator Z into one shared PSUM pair-bank.  AV(mt-1) is emitted after
   scores(mt) (software pipeline) and pss bufs=3 gives the scores one
   exp of lookahead, so ACT stays ~95% busy.
 - U|Z evicted per pair with one DVE copy; partition-base remaps ride
   small SBUF->SBUF DMAs on the scalar/gpsimd queues; 1/Z via one DVE
   reciprocal_approx_fast per half; attn_r = U * (1/Z) on DVE (f32r).
 - Dense full-array filler matmuls bridge the eviction/normalize tail so
   HAM stays warm into the MLP.
 - Depthwise 3x3 runs on the PE as 9 diagonal-matrix taps over an x-padded
   [32, 36] layout; each tap's diagonal splits into four independent
   [32,32] blocks at tile_position (32i,32i) running concurrently on the
   PE sub-array grid (~4x).  All diag weights are SBUF-resident.
 - gelu is emitted as bf16 straight into the fc2 moving operand; fc2 runs
   in bf16.  Small bias vectors ride one consolidated "smalls" DMA.
"""
import numpy as np

import concourse.bass as bass
import concourse.tile as tile
import concourse.mybir as mybir
from concourse import bacc
from concourse.bass_utils import run_bass_kernel_spmd

F32 = mybir.dt.float32
F32R = mybir.dt.float32r
BF16 = mybir.dt.bfloat16
FP16 = mybir.dt.float16
AF = mybir.ActivationFunctionType
ALU = mybir.AluOpType

B, C, H, W = 8, 256, 32, 32
NH, DH = 8, 32          # heads, head dim
HW = H * W              # 1024 positions
HID = 4 * C             # 1024 mlp hidden channels
EPS = 1e-5
WP = W + 4              # x-padded row width (36, even)
PADF = H * WP           # padded flat spatial size (1152)
N_CORES = 8

_NC_CACHE = {}

import concourse.bass_utils as _bu

if not getattr(_bu, "_ldwopt_patched", False):
    _orig_run_command = _bu.run_command

    def _run_command_ldwopt(cmd, **kw):
        cmd = list(cmd)  # ldw-opt incompatible with explicit InstLdweights
        return _orig_run_command(cmd, **kw)

    _bu.run_command = _run_command_ldwopt
    _bu._ldwopt_patched = True


def _tap_chunks(shift):
    """Bank-aligned (<=512) even-aligned chunks of a dw tap's dst range.

    Even dst offsets/counts keep every chunk bank-friendly; the elements
    dropped by the even-alignment are always x-pad columns (never read
    downstream).
    """
    lo, hi = max(0, -shift), min(PADF, PADF - shift)
    out = []
    for b0 in range(0, PADF, 512):
        a, b = max(lo, b0), min(hi, b0 + 512)
        a += a % 2
        n = (b - a) & ~1
        if n > 0:
            out.append((a, n))
    return out


def _build_body(nc, tc, io):
    x1d, x2d = io["x1"], io["x2"]
    outd = io["out"]

    import contextlib
    ctx = contextlib.ExitStack()
    with ctx:
        wpool = ctx.enter_context(tc.tile_pool(name="weights", bufs=1))
        # pB: attention results that survive until after proj
        pB = ctx.enter_context(tc.tile_pool(name="pB", bufs=1))

        # ---------- persistent SBUF tensors ----------
        x1 = wpool.tile([128, 2, HW], F32R, tag="x1")
        x2w = wpool.tile([128, 2, HW], F32R, tag="x2w")
        nc.sync.dma_start(x1[:], x1d[:])
        nc.sync.dma_start(x2w[:], x2d[:])

        def wload(name, shape, dt):
            t = wpool.tile(shape, dt, tag=name)
            nc.sync.dma_start(t[:], io[name][:])
            return t

        # attention weights first: the sync DMA queue is FIFO and the qk
        # matmuls are the head of the critical path
        qT = wload("qT", [128, 2, C], F32R)
        kT = wload("kT", [128, 2, C], F32R)
        smalls = wload("smalls", [128, 44], F32)
        qb, kb = smalls[:, 0:2], smalls[:, 2:4]
        inv1, beta1 = smalls[:, 4:6], smalls[:, 6:8]
        projb, fc2b = smalls[:, 8:10], smalls[:, 10:12]
        fc1b = smalls[:, 12:20]
        dwc = smalls[:, 20:28]   # depthwise center-tap weights
        dwn = smalls[:, 28:36]   # (dy=-1, dx=0) tap weights
        dws = smalls[:, 36:44]   # (dy=+1, dx=0) tap weights
        vwT = wload("vwT", [128, 2, C], F32R)

        attnU = pB.tile([128, 2, HW], BF16, tag="attnU")   # unnormalized attn
        zc = pB.tile([128, 2, HW], BF16, tag="zc")         # Z at U-aligned rows
        attn_r = pB.tile([128, 2, HW], F32R, tag="attn_r")
        rbc = pB.tile([128, 2, HW], F32, tag="rbc")
        zf = pB.tile([128, 2, HW], F32, tag="zf")

        with tc.tile_pool(name="pA", bufs=1) as pA:
            q_sb = pA.tile([128, 2, HW], BF16, tag="q")

            k_sb = pA.tile([128, 2, HW], BF16, tag="k")
            # AV stationary per (mt, h): [128, 64] = [v(32) | ones(32)]
            vt1 = pA.tile([128, 8, NH, 2 * DH], BF16, tag="vt1")
            nc.gpsimd.memset(vt1[:], 1.0)


            # preload the exp table set while DMAs stream in
            warm = pA.tile([1, 2], F32, tag="warm")
            nc.scalar.activation(warm[:], inv1[0:1, 0:2], AF.Exp)

            # ---------- phase 1: q, k projections; x2n; v^T ----------
            with tc.tile_pool(name="p1", bufs=1) as p1, \
                 tc.tile_pool(name="ps1", bufs=2, space="PSUM") as ps1, tc.tile_pool(name="psv", bufs=3, space="PSUM") as psv_pool:
                # MLP weights: behind x2 on the queue — needed much later
                projT = wload("projT", [128, 2, C], F32R)
                fc1T = wload("fc1T", [128, 2, HID], F32R)
                fc2T = wload("fc2T", [128, 8, C], BF16)
                dwd = wload("dwd", [128, 8, 9, 128], FP16)
                def qk_half(kt):
                    for (wT, bias, dst) in ((qT, qb, q_sb), (kT, kb, k_sb)):
                        rhs = x1 if dst is q_sb else x2w
                        ps = ps1.tile([128, HW], F32, tag="mm",
                                      name=f"qk_{kt}_{0 if dst is q_sb else 1}")
                        for kin in range(2):
                            for chk in range(2):
                                nc.tensor.matmul(
                                    ps[:, 512 * chk:512 * (chk + 1)],
                                    wT[:, kin, 128 * kt:128 * (kt + 1)],
                                    rhs[:, kin, 512 * chk:512 * (chk + 1)],
                                    start=(kin == 0), stop=(kin == 1))
                        nc.vector.tensor_scalar_add(
                            dst[:, kt, :], ps[:], bias[:, kt:kt + 1])

                qk_half(0)      # half-0 scores only need this + v

                x2n = p1.tile([128, 2, HW], F32R, tag="x2n")
                for kin in range(2):
                    nc.vector.tensor_scalar(
                        x2n[:, kin, :], x2w[:, kin, :],
                        inv1[:, kin:kin + 1], beta1[:, kin:kin + 1],
                        ALU.mult, ALU.add)

                for mp in range(8):
                    psv = psv_pool.tile([128, C], F32, tag="mmv", name=f"v_{mp}")
                    for kin in range(2):
                        nc.tensor.matmul(
                            psv[:], x2n[:, kin, 128 * mp:128 * (mp + 1)],
                            vwT[:, kin, :], start=(kin == 0), stop=(kin == 1))
                    # [128, 256] -> v columns of all 8 heads for this m-tile
                    nc.vector.tensor_copy(
                        vt1[:, mp, :, 0:DH],
                        psv[:].rearrange("p (h d) -> p h d", h=NH))


                qk_half(1)

            # ---------- phase 2: attention ----------
            # heads in pair order (A, B): A writes PSUM partitions 0..63,
            # B (PE column tiling) writes 64..127 of the same pair bank.
            with tc.tile_pool(name="expS", bufs=6) as xpool, \
                 tc.tile_pool(name="ev", bufs=4) as evpool, \
                 tc.tile_pool(name="pss", bufs=3, space="PSUM") as pss, \
                 tc.tile_pool(name="psa", bufs=1, space="PSUM") as psa:
                for half in range(2):
                    for pr in range(2):          # pairs (0,2) and (1,3)
                        pair = [4 * half + pr, 4 * half + pr + 2]
                        hbs = [32 * pr, 32 * pr + 64]   # k/q row groups
                        ubs = [0, 64]                    # U base in pair bank
                        ps_av = psa.tile([128, HW], F32, tag="av",
                                         name=f"av_{half}_{pr}")
                        pend = None      # exp tiles awaiting their AV matmuls
                        for mt in range(8):
                            # both members' scores run row-concurrent on
                            # distinct 32-row PE groups
                            scs = []
                            for mi in range(2):
                                scs.append(pss.tile(
                                    [128, HW], F32, tag="s",
                                    name=f"s_{half}_{pr}_{mt}_{mi}"))
                            for chk in range(2):
                                for mi in range(2):
                                    hb = hbs[mi]
                                    nc.tensor.matmul(
                                        scs[mi][:, 512 * chk:512 * (chk + 1)],
                                        k_sb[hb:hb + 32, half,
                                             128 * mt:128 * (mt + 1)],
                                        q_sb[hb:hb + 32, half,
                                             512 * chk:512 * (chk + 1)],
                                        start=True, stop=True,
                                        tile_position=(hb, 0))
                            exs = []
                            for mi in range(2):
                                ex = xpool.tile([128, HW], BF16, tag="expS")
                                nc.scalar.activation(ex[:], scs[mi][:],
                                                     AF.Exp)
                                exs.append(ex)
                            # software pipeline: AV(mt-1) is emitted after
                            # scores(mt), so the PE never stalls on ACT
                            def av(emt, exs_):
                                for chk in range(2):
                                    for mi in range(2):
                                        ub = ubs[mi]
                                        nc.tensor.matmul(
                                            ps_av[ub:ub + 64,
                                                  512 * chk:512 * (chk + 1)],
                                            vt1[:, emt, pair[mi], :],
                                            exs_[mi][:, 512 * chk:512 * (chk + 1)],
                                            start=(emt == 0), stop=(emt == 7),
                                            tile_position=(0, ub))
                            if pend is not None:
                                av(mt - 1, pend)
                            pend = exs
                        av(7, pend)
                        # evict U|Z together ([64,1024] per member), then
                        # redistribute by small SBUF->SBUF DMAs: U to
                        # attnU rows 32*hl, Z to zc rows 32*hl (same rows).
                        uzt = evpool.tile([128, HW], BF16, tag="uzt")
                        nc.vector.tensor_copy(uzt[:], ps_av[:])
                        for mi, h in enumerate(pair):
                            hl, ub = h % 4, ubs[mi]
                            if hl % 2 == 0:
                                # U rows already in place; Z shifts down 32
                                nc.vector.tensor_copy(
                                    attnU[ub:ub + 32, half, :],
                                    uzt[ub:ub + 32, :])
                                nc.scalar.dma_start(
                                    zc[ub:ub + 32, half, :],
                                    uzt[ub + 32:ub + 64, :])
                            else:
                                # Z rows already in place; U shifts up 32
                                nc.vector.tensor_copy(
                                    zc[ub + 32:ub + 64, half, :],
                                    uzt[ub + 32:ub + 64, :])
                                nc.gpsimd.dma_start(
                                    attnU[ub + 32:ub + 64, half, :],
                                    uzt[ub:ub + 32, :])
                    # normalize this half as soon as its pairs are done
                    nc.vector.tensor_copy(zf[:, half, :], zc[:, half, :])
                    nc.vector.reciprocal_approx_fast(rbc[:, half, :],
                                                     zf[:, half, :])
                    nc.vector.tensor_mul(attn_r[:, half, :],
                                         attnU[:, half, :], rbc[:, half, :])
                if half == 1:
                    # HAM keep-alive: dense full-array matmuls bridge the
                    # eviction/normalize tail so the MLP starts at 2.4 GHz
                    dmy = pss.tile([128, HW], F32, tag="s", name="dmy")
                    for i in range(8):
                        for chk in range(2):
                            nc.tensor.matmul(
                                dmy[:, 512 * chk:512 * (chk + 1)],
                                qT[:, 0, 0:128], x1[:, 0, 512 * chk:512 * (chk + 1)],
                                start=True, stop=True)

        # ---------- phase 3: proj, residual, MLP ----------
        with tc.tile_pool(name="pC", bufs=1) as pC:
            x1u = pC.tile([128, 2, HW], F32R, tag="x1u")
            h1 = pC.tile([128, 8, H, WP], FP16, tag="h1")
            # zero the x-pad columns
            zpad = pC.tile([128, 8 * H * 2], F32, tag="zpad")
            nc.gpsimd.memset(zpad[:], 0.0)
            zsrc = zpad[:].rearrange("p (c a b) -> p c a b", c=8, a=H)
            nc.vector.tensor_copy(h1[:, :, :, 0:2], zsrc)
            nc.vector.tensor_copy(h1[:, :, :, WP - 2:WP], zsrc)
            hgr = pC.tile([128, 8, HW], BF16, tag="hgr")
            out_sb = pC.tile([128, 2, HW], F32, tag="out")

            # proj + fc1 get a deep psum pool (dense PE stream warms HAM)
            with tc.tile_pool(name="ps2a", bufs=3, space="PSUM") as ps2a:
                # proj + residual1
                for mt in range(2):
                    pp = ps2a.tile([128, HW], F32, tag="mf", name=f"pj_{mt}")
                    for kt in range(2):
                        for chk in range(2):
                            nc.tensor.matmul(
                                pp[:, 512 * chk:512 * (chk + 1)],
                                projT[:, kt, 128 * mt:128 * (mt + 1)],
                                attn_r[:, kt, 512 * chk:512 * (chk + 1)],
                                start=(kt == 0), stop=(kt == 1))
                    nc.vector.scalar_tensor_tensor(
                        x1u[:, mt, :], pp[:],
                        projb[:, mt:mt + 1],
                        x1[:, mt, :],
                        ALU.add, ALU.add)

                # ---------- phase 4: MLP ----------
                for mt in range(8):
                    pf = ps2a.tile([128, HW], F32, tag="mf",
                                   name=f"f1_{mt}")
                    for kt in range(2):
                        for chk in range(2):
                            nc.tensor.matmul(
                                pf[:, 512 * chk:512 * (chk + 1)],
                                fc1T[:, kt, 128 * mt:128 * (mt + 1)],
                                x1u[:, kt, 512 * chk:512 * (chk + 1)],
                                start=(kt == 0), stop=(kt == 1))
                    nc.scalar.activation(
                        h1[:, mt, :, 2:W + 2],
                        pf[:].rearrange("p (a b) -> p a b", a=H),
                        AF.Identity, bias=fc1b[:, mt:mt + 1])

            # depthwise 3x3: 9 diagonal taps; each tap's diagonal splits into
            # four independent [32,32] blocks at tile_position (32i,32i) that
            # run concurrently on the PE's sub-array grid.
            h1f = h1[:].rearrange("p c a b -> p c (a b)")
            # center tap runs on the DVE (aligned fp16 multiply + merge);
            # the PE handles the 8 shifted taps
            taps = [(dy, dx) for dy in (-1, 0, 1) for dx in (-1, 0, 1)]
            taps.remove((0, 0))
            with tc.tile_pool(name="psd", bufs=2, space="PSUM") as psd, \
                 tc.tile_pool(name="ps2b", bufs=1, space="PSUM") as ps2b:
                for ct in range(8):
                    ps_dw = psd.tile([128, PADF], F32, tag="dw")
                    tc_t = pC.tile([128, H, W], FP16, tag="tct",
                                   name=f"tct_{ct % 2}")
                    nc.vector.tensor_scalar_mul(
                        tc_t[:], h1[:, ct, :, 2:W + 2], dwc[:, ct:ct + 1])
                    for ti, (dy, dx) in enumerate(taps):
                        shift = dy * WP + dx
                        chunks = _tap_chunks(shift)
                        for ci, (c0, n) in enumerate(chunks):
                            for qi in range(4):
                                nc.tensor.matmul(
                                    ps_dw[32 * qi:32 * qi + 32, c0:c0 + n],
                                    dwd[32 * qi:32 * qi + 32, ct,
                                        3 * (dy + 1) + (dx + 1),
                                        32 * qi:32 * qi + 32],
                                    h1f[32 * qi:32 * qi + 32, ct,
                                        c0 + shift:c0 + shift + n],
                                    start=(ti == 0),
                                    stop=(ti == len(taps) - 1
                                          and ci == len(chunks) - 1),
                                    tile_position=(32 * qi, 32 * qi))
                    ctr = ps_dw[:].rearrange("p (a b) -> p a b",
                                             a=H)[:, :, 2:W + 2]
                    nc.vector.tensor_add(ctr, tc_t[:], ctr)
                    nc.scalar.activation(hgr[:, ct, :], ctr, AF.Gelu)

                # fc2 + residual2
                for mt in range(2):
                    pg = ps2b.tile([128, 512], F32, tag="mm",
                                   name=f"f2_{mt}_0")
                    pg2 = ps2b.tile([128, 512], F32, tag="mm2",
                                    name=f"f2_{mt}_1")
                    pgs = [pg, pg2]
                    for kt in range(8):
                        for chk in range(2):
                            nc.tensor.matmul(
                                pgs[chk][:], fc2T[:, kt, 128 * mt:128 * (mt + 1)],
                                hgr[:, kt, 512 * chk:512 * (chk + 1)],
                                start=(kt == 0), stop=(kt == 7))
                    for chk in range(2):
                        nc.vector.scalar_tensor_tensor(
                            out_sb[:, mt, 512 * chk:512 * (chk + 1)], pgs[chk][:],
                            fc2b[:, mt:mt + 1],
                            x1u[:, mt, 512 * chk:512 * (chk + 1)],
                            ALU.add, ALU.add)
                    nc.sync.dma_start(outd[:, mt, :], out_sb[:, mt, :])


def _build_nc():
    if "nc" in _NC_CACHE:
        return _NC_CACHE["nc"]
    nc = bacc.Bacc(trn_type="TRN2", target_bir_lowering=False, debug=False)
    io = {}
    for name, shape, dt in [
        ("x1", [128, 2, HW], F32R), ("x2", [128, 2, HW], F32R),
        ("qT", [128, 2, C], F32R), ("kT", [128, 2, C], F32R),
        ("vwT", [128, 2, C], F32R), ("projT", [128, 2, C], F32R),
        ("fc1T", [128, 2, HID], F32R), ("fc2T", [128, 8, C], BF16),
        ("dwd", [128, 8, 9, 128], FP16),
        ("smalls", [128, 44], F32),
    ]:
        io[name] = nc.dram_tensor(name, shape, dt, kind="ExternalInput").ap()
    io["out"] = nc.dram_tensor("out", [128, 2, HW], F32, kind="ExternalOutput").ap()

    with tile.TileContext(nc) as tc:
        _build_body(nc, tc, io)
    nc.compile()
    _NC_CACHE["nc"] = nc
    return nc


def _to_part_layout(a, ntiles):
    """[ntiles*128, F] -> [128, ntiles, F] with c = kt*128 + p."""
    return np.ascontiguousarray(
        a.reshape(ntiles, 128, -1).transpose(1, 0, 2))


def _bias_layout(b, ntiles):
    """[ntiles*128] -> [128, ntiles]."""
    return np.ascontiguousarray(b.reshape(ntiles, 128).T)


def _prepare_weights(bn1_g, bn1_b, bn1_m, bn1_v, q_w, k_w, v_w, temp, proj_w,
                     proj_b, bn2_g, bn2_b, bn2_m, bn2_v, fc1_w, fc1_b, dw_w,
                     fc2_w, fc2_b):
    f64 = np.float64
    import ml_dtypes
    bf16 = ml_dtypes.bfloat16

    inv1 = (bn1_g.astype(f64) / np.sqrt(bn1_v.astype(f64) + EPS))
    beta1 = bn1_b.astype(f64) - bn1_m.astype(f64) * inv1
    inv2 = (bn2_g.astype(f64) / np.sqrt(bn2_v.astype(f64) + EPS))
    beta2 = bn2_b.astype(f64) - bn2_m.astype(f64) * inv2

    tscale = np.repeat(temp.astype(f64), DH)                     # [256]
    qw_f = q_w.astype(f64) * inv1[None, :] * tscale[:, None]
    qb = (q_w.astype(f64) @ beta1) * tscale
    kw_f = k_w.astype(f64) * inv1[None, :]
    kb = k_w.astype(f64) @ beta1
    fc1w_f = fc1_w.astype(f64) * inv2[None, :]
    fc1bf = fc1_b.astype(f64) + fc1_w.astype(f64) @ beta2

    # diag tap matrices, [k_row, ct, tap, m_col] so SBUF partition dim is k
    dwd = np.zeros((8, 9, 128, 128), np.float32)
    idx = np.arange(128)
    for ct in range(8):
        for t in range(9):
            dy, dx = t // 3, t % 3
            dwd[ct, t, idx, idx] = dw_w[ct * 128 + idx, 0, dy, dx]
    dwd = np.ascontiguousarray(dwd.transpose(2, 0, 1, 3))

    smalls = np.concatenate([
        _bias_layout(qb.astype(np.float32), 2),
        _bias_layout(kb.astype(np.float32), 2),
        _bias_layout(inv1.astype(np.float32), 2),
        _bias_layout(beta1.astype(np.float32), 2),
        _bias_layout(proj_b.astype(np.float32), 2),
        _bias_layout(fc2_b.astype(np.float32), 2),
        _bias_layout(fc1bf.astype(np.float32), 8),
        _bias_layout(dw_w[:, 0, 1, 1].astype(np.float32), 8),
        _bias_layout(dw_w[:, 0, 0, 1].astype(np.float32), 8),
        _bias_layout(dw_w[:, 0, 2, 1].astype(np.float32), 8),
    ], axis=1)
    w = {
        "qT": _to_part_layout(np.ascontiguousarray(qw_f.T).astype(np.float32), 2),
        "kT": _to_part_layout(np.ascontiguousarray(kw_f.T).astype(np.float32), 2),
        "vwT": _to_part_layout(np.ascontiguousarray(v_w.T).astype(np.float32), 2),
        "projT": _to_part_layout(np.ascontiguousarray(proj_w.T).astype(np.float32), 2),
        "fc1T": _to_part_layout(np.ascontiguousarray(fc1w_f.T).astype(np.float32), 2),
        "fc2T": _to_part_layout(np.ascontiguousarray(fc2_w.T).astype(np.float32), 8).astype(bf16),
        "dwd": dwd.astype(np.float16),
        "smalls": np.ascontiguousarray(smalls),
    }
    return w


_LAST_RESULTS = {}


def kernel(x1, x2, bn1_g, bn1_b, bn1_m, bn1_v, q_w, k_w, v_w, temp, proj_w,
           proj_b, bn2_g, bn2_b, bn2_m, bn2_v, fc1_w, fc1_b, dw_w, fc2_w,
           fc2_b, _trace=False):
    x1 = np.asarray(x1, np.float32)
    x2 = np.asarray(x2, np.float32)
    args = [np.asarray(a) for a in
            (bn1_g, bn1_b, bn1_m, bn1_v, q_w, k_w, v_w, temp, proj_w, proj_b,
             bn2_g, bn2_b, bn2_m, bn2_v, fc1_w, fc1_b, dw_w, fc2_w, fc2_b)]
    w = _prepare_weights(*args)

    nc = _build_nc()
    in_maps = []
    for i in range(N_CORES):
        m = dict(w)
        m["x1"] = _to_part_layout(x1[i].reshape(C, HW), 2)
        m["x2"] = _to_part_layout(x2[i].reshape(C, HW), 2)
        in_maps.append(m)

    res = run_bass_kernel_spmd(nc, in_maps, core_ids=list(range(N_CORES)),
                               trace=_trace)
    _LAST_RESULTS["res"] = res

    out = np.empty((B, C, H, W), np.float32)
    for i in range(N_CORES):
        o = res.results[i]["out"]                    # [128, 2, 1024]
        out[i] = o.transpose(1, 0, 2).reshape(C, H, W)
    return out


# revision 42
# speedup vs baseline: 1.0568x; 1.0568x over previous
"""CMXBlock (dense transformer block) Trainium2 Bass kernel.

Sharding: data-parallel over batch B=8 across the 8 NeuronCores — one image
per core, all weights replicated, no collectives.

Per-core computation (C=256 channels on partitions, HW=1024 positions free):
  x1 <- x1 + proj(softmax((q_w@bn1(x1))^T (k_w@bn1(x2)) * temp) @ (v_w@bn1(x2))^T)
  x1 <- x1 + fc2(gelu(dwconv3x3(fc1(bn2(x1)))))

The kernel is ACT(exp)-bound in attention and PE-bound in the MLP; the
HAM clock gate never warms during attention (small-K scores / M=64 AV do
not register as PE activity), so the attention pipeline is organized to
hide the 1.2 GHz PE entirely under the exp stream.

Key structure:
 - BatchNorms/temp folded into conv weights host-side; per-partition biases
   applied at PSUM eviction.  q/k in bf16, scores K=32 f32 PSUM.
 - Attention processes head pairs (h, h+2) with members interleaved per
   m-tile: scores run row-concurrent on PE row groups (hb, hb+64), the exp
   of each [128,1024] score tile goes ACT->bf16 SBUF directly, and the AV
   matmuls of the two members run column-concurrent (M=64 stationaries
   [v | ones] at PE columns 0/64) accumulating U and the softmax
   denominator Z into one shared PSUM pair-bank.  AV(mt-1) is emitted after
   scores(mt) (software pipeline) and pss bufs=3 gives the scores one
   exp of lookahead, so ACT stays ~95% busy.
 - U|Z evicted per pair with one DVE copy; partition-base remaps ride
   small SBUF->SBUF DMAs on the scalar/gpsimd queues; 1/Z via one DVE
   reciprocal_approx_fast per half; attn_r = U * (1/Z) on DVE (f32r).
 - Dense full-array filler matmuls bridge the eviction/normalize tail so
   HAM stays warm into the MLP.
 - Depthwise 3x3 runs on the PE as 9 diagonal-matrix taps over an x-padded
   [32, 36] layout; each tap's diagonal splits into four independent
   [32,32] blocks at tile_position (32i,32i) running concurrently on the
   PE sub-array grid (~4x).  All diag weights are SBUF-resident.
 - gelu is emitted as bf16 straight into the fc2 moving operand; fc2 runs
   in bf16.  Small bias vectors ride one consolidated "smalls" DMA.
"""
import numpy as np

import concourse.bass as bass
import concourse.tile as tile
import concourse.mybir as mybir
from concourse import bacc
from concourse.bass_utils import run_bass_kernel_spmd

F32 = mybir.dt.float32
F32R = mybir.dt.float32r
BF16 = mybir.dt.bfloat16
FP16 = mybir.dt.float16
AF = mybir.ActivationFunctionType
ALU = mybir.AluOpType

B, C, H, W = 8, 256, 32, 32
NH, DH = 8, 32          # heads, head dim
HW = H * W              # 1024 positions
HID = 4 * C             # 1024 mlp hidden channels
EPS = 1e-5
WP = W + 4              # x-padded row width (36, even)
PADF = H * WP           # padded flat spatial size (1152)
N_CORES = 8

_NC_CACHE = {}

import concourse.bass_utils as _bu

if not getattr(_bu, "_ldwopt_patched", False):
    _orig_run_command = _bu.run_command

    def _run_command_ldwopt(cmd, **kw):
        cmd = list(cmd)  # ldw-opt incompatible with explicit InstLdweights
        return _orig_run_command(cmd, **kw)

    _bu.run_command = _run_command_ldwopt
    _bu._ldwopt_patched = True


def _tap_chunks(shift):
    """Bank-aligned (<=512) even-aligned chunks of a dw tap's dst range.

    Even dst offsets/counts keep every chunk bank-friendly; the elements
    dropped by the even-alignment are always x-pad columns (never read
    downstream).
    """
    lo, hi = max(0, -shift), min(PADF, PADF - shift)
    out = []
    for b0 in range(0, PADF, 512):
        a, b = max(lo, b0), min(hi, b0 + 512)
        a += a % 2
        n = (b - a) & ~1
        if n > 0:
            out.append((a, n))
    return out


def _build_body(nc, tc, io):
    x1d, x2d = io["x1"], io["x2"]
    outd = io["out"]

    import contextlib
    ctx = contextlib.ExitStack()
    with ctx:
        wpool = ctx.enter_context(tc.tile_pool(name="weights", bufs=1))
        # pB: attention results that survive until after proj
        pB = ctx.enter_context(tc.tile_pool(name="pB", bufs=1))

        # ---------- persistent SBUF tensors ----------
        x1 = wpool.tile([128, 2, HW], F32R, tag="x1")
        x2w = wpool.tile([128, 2, HW], F32R, tag="x2w")
        nc.sync.dma_start(x1[:], x1d[:])
        nc.sync.dma_start(x2w[:], x2d[:])

        def wload(name, shape, dt):
            t = wpool.tile(shape, dt, tag=name)
            nc.sync.dma_start(t[:], io[name][:])
            return t

        # attention weights first: the sync DMA queue is FIFO and the qk
        # matmuls are the head of the critical path
        qT = wload("qT", [128, 2, C], F32R)
        kT = wload("kT", [128, 2, C], F32R)
        smalls = wload("smalls", [128, 44], F32)
        qb, kb = smalls[:, 0:2], smalls[:, 2:4]
        inv1, beta1 = smalls[:, 4:6], smalls[:, 6:8]
        projb, fc2b = smalls[:, 8:10], smalls[:, 10:12]
        fc1b = smalls[:, 12:20]
        dwc = smalls[:, 20:28]   # depthwise center-tap weights
        dwn = smalls[:, 28:36]   # (dy=-1, dx=0) tap weights
        dws = smalls[:, 36:44]   # (dy=+1, dx=0) tap weights
        vwT = wload("vwT", [128, 2, C], F32R)

        attnU = pB.tile([128, 2, HW], BF16, tag="attnU")   # unnormalized attn
        zc = pB.tile([128, 2, HW], BF16, tag="zc")         # Z at U-aligned rows
        attn_r = pB.tile([128, 2, HW], F32R, tag="attn_r")
        rbc = pB.tile([128, 2, HW], F32, tag="rbc")
        zf = pB.tile([128, 2, HW], F32, tag="zf")

        with tc.tile_pool(name="pA", bufs=1) as pA:
            q_sb = pA.tile([128, 2, HW], BF16, tag="q")

            k_sb = pA.tile([128, 2, HW], BF16, tag="k")
            # AV stationary per (mt, h): [128, 64] = [v(32) | ones(32)]
            vt1 = pA.tile([128, 8, NH, 2 * DH], BF16, tag="vt1")
            nc.gpsimd.memset(vt1[:], 1.0)


            # preload the exp table set while DMAs stream in
            warm = pA.tile([1, 2], F32, tag="warm")
            nc.scalar.activation(warm[:], inv1[0:1, 0:2], AF.Exp)

            # ---------- phase 1: q, k projections; x2n; v^T ----------
            with tc.tile_pool(name="p1", bufs=1) as p1, \
                 tc.tile_pool(name="ps1", bufs=2, space="PSUM") as ps1, tc.tile_pool(name="psv", bufs=3, space="PSUM") as psv_pool:
                # MLP weights: behind x2 on the queue — needed much later
                projT = wload("projT", [128, 2, C], F32R)
                fc1T = wload("fc1T", [128, 2, HID], F32R)
                fc2T = wload("fc2T", [128, 8, C], BF16)
                dwd = wload("dwd", [128, 8, 9, 128], FP16)
                def qk_half(kt):
                    for (wT, bias, dst) in ((qT, qb, q_sb), (kT, kb, k_sb)):
                        rhs = x1 if dst is q_sb else x2w
                        ps = ps1.tile([128, HW], F32, tag="mm",
                                      name=f"qk_{kt}_{0 if dst is q_sb else 1}")
                        for kin in range(2):
                            for chk in range(2):
                                nc.tensor.matmul(
                                    ps[:, 512 * chk:512 * (chk + 1)],
                                    wT[:, kin, 128 * kt:128 * (kt + 1)],
                                    rhs[:, kin, 512 * chk:512 * (chk + 1)],
                                    start=(kin == 0), stop=(kin == 1))
                        nc.vector.tensor_scalar_add(
                            dst[:, kt, :], ps[:], bias[:, kt:kt + 1])

                qk_half(0)      # half-0 scores only need this + v

                x2n = p1.tile([128, 2, HW], F32R, tag="x2n")
                for kin in range(2):
                    nc.vector.tensor_scalar(
                        x2n[:, kin, :], x2w[:, kin, :],
                        inv1[:, kin:kin + 1], beta1[:, kin:kin + 1],
                        ALU.mult, ALU.add)

                for mp in range(8):
                    psv = psv_pool.tile([128, C], F32, tag="mmv", name=f"v_{mp}")
                    for kin in range(2):
                        nc.tensor.matmul(
                            psv[:], x2n[:, kin, 128 * mp:128 * (mp + 1)],
                            vwT[:, kin, :], start=(kin == 0), stop=(kin == 1))
                    # [128, 256] -> v columns of all 8 heads for this m-tile
                    nc.vector.tensor_copy(
                        vt1[:, mp, :, 0:DH],
                        psv[:].rearrange("p (h d) -> p h d", h=NH))


                qk_half(1)

            # ---------- phase 2: attention ----------
            # heads in pair order (A, B): A writes PSUM partitions 0..63,
            # B (PE column tiling) writes 64..127 of the same pair bank.
            with tc.tile_pool(name="expS", bufs=6) as xpool, \
                 tc.tile_pool(name="ev", bufs=4) as evpool, \
                 tc.tile_pool(name="pss", bufs=3, space="PSUM") as pss, \
                 tc.tile_pool(name="psa", bufs=1, space="PSUM") as psa:
                for half in range(2):
                    for pr in range(2):          # pairs (0,2) and (1,3)
                        pair = [4 * half + pr, 4 * half + pr + 2]
                        hbs = [32 * pr, 32 * pr + 64]   # k/q row groups
                        ubs = [0, 64]                    # U base in pair bank
                        ps_av = psa.tile([128, HW], F32, tag="av",
                                         name=f"av_{half}_{pr}")
                        pend = None      # exp tiles awaiting their AV matmuls
                        for mt in range(8):
                            # both members' scores run row-concurrent on
                            # distinct 32-row PE groups
                            scs = []
                            for mi in range(2):
                                scs.append(pss.tile(
                                    [128, HW], F32, tag="s",
                                    name=f"s_{half}_{pr}_{mt}_{mi}"))
                            for chk in range(2):
                                for mi in range(2):
                                    hb = hbs[mi]
                                    nc.tensor.matmul(
                                        scs[mi][:, 512 * chk:512 * (chk + 1)],
                                        k_sb[hb:hb + 32, half,
                                             128 * mt:128 * (mt + 1)],
                                        q_sb[hb:hb + 32, half,
                                             512 * chk:512 * (chk + 1)],
                                        start=True, stop=True,
                                        tile_position=(hb, 0))
                            exs = []
                            for mi in range(2):
                                ex = xpool.tile([128, HW], BF16, tag="expS")
                                nc.scalar.activation(ex[:], scs[mi][:],
                                                     AF.Exp)
                                exs.append(ex)
                            # software pipeline: AV(mt-1) is emitted after
                            # scores(mt), so the PE never stalls on ACT
                            def av(emt, exs_):
                                for chk in range(2):
                                    for mi in range(2):
                                        ub = ubs[mi]
                                        nc.tensor.matmul(
                                            ps_av[ub:ub + 64,
                                                  512 * chk:512 * (chk + 1)],
                                            vt1[:, emt, pair[mi], :],
                                            exs_[mi][:, 512 * chk:512 * (chk + 1)],
                                            start=(emt == 0), stop=(emt == 7),
                                            tile_position=(0, ub))
                            if pend is not None:
                                av(mt - 1, pend)
                            pend = exs
                        av(7, pend)
                        # evict U|Z together ([64,1024] per member), then
                        # redistribute by small SBUF->SBUF DMAs: U to
                        # attnU rows 32*hl, Z to zc rows 32*hl (same rows).
                        uzt = evpool.tile([128, HW], BF16, tag="uzt")
                        nc.vector.tensor_copy(uzt[:], ps_av[:])
                        for mi, h in enumerate(pair):
                            hl, ub = h % 4, ubs[mi]
                            if hl % 2 == 0:
                                # U rows already in place; Z shifts down 32
                                nc.vector.tensor_copy(
                                    attnU[ub:ub + 32, half, :],
                                    uzt[ub:ub + 32, :])
                                nc.scalar.dma_start(
                                    zc[ub:ub + 32, half, :],
                                    uzt[ub + 32:ub + 64, :])
                            else:
                                # Z rows already in place; U shifts up 32
                                nc.vector.tensor_copy(
                                    zc[ub + 32:ub + 64, half, :],
                                    uzt[ub + 32:ub + 64, :])
                                nc.gpsimd.dma_start(
                                    attnU[ub + 32:ub + 64, half, :],
                                    uzt[ub:ub + 32, :])
                    # normalize this half as soon as its pairs are done
                    nc.vector.tensor_copy(zf[:, half, :], zc[:, half, :])
                    nc.vector.reciprocal_approx_fast(rbc[:, half, :],
                                                     zf[:, half, :])
                    nc.vector.tensor_mul(attn_r[:, half, :],
                                         attnU[:, half, :], rbc[:, half, :])
                if half == 1:
                    # HAM keep-alive: dense full-array matmuls bridge the
                    # eviction/normalize tail so the MLP starts at 2.4 GHz
                    dmy = pss.tile([128, HW], F32, tag="s", name="dmy")
                    for i in range(8):
                        for chk in range(2):
                            nc.tensor.matmul(
                                dmy[:, 512 * chk:512 * (chk + 1)],
                                qT[:, 0, 0:128], x1[:, 0, 512 * chk:512 * (chk + 1)],
                                start=True, stop=True)

        # ---------- phase 3: proj, residual, MLP ----------
        with tc.tile_pool(name="pC", bufs=1) as pC:
            x1u = pC.tile([128, 2, HW], F32R, tag="x1u")
            h1 = pC.tile([128, 8, H, WP], FP16, tag="h1")
            # zero the x-pad columns
            zpad = pC.tile([128, 8 * H * 2], F32, tag="zpad")
            nc.gpsimd.memset(zpad[:], 0.0)
            zsrc = zpad[:].rearrange("p (c a b) -> p c a b", c=8, a=H)
            nc.vector.tensor_copy(h1[:, :, :, 0:2], zsrc)
            nc.vector.tensor_copy(h1[:, :, :, WP - 2:WP], zsrc)
            hgr = pC.tile([128, 8, HW], BF16, tag="hgr")
            out_sb = pC.tile([128, 2, HW], F32, tag="out")

            # proj + fc1 get a deep psum pool (dense PE stream warms HAM)
            with tc.tile_pool(name="ps2a", bufs=3, space="PSUM") as ps2a:
                # proj + residual1
                for mt in range(2):
                    pp = ps2a.tile([128, HW], F32, tag="mf", name=f"pj_{mt}")
                    for kt in range(2):
                        for chk in range(2):
                            nc.tensor.matmul(
                                pp[:, 512 * chk:512 * (chk + 1)],
                                projT[:, kt, 128 * mt:128 * (mt + 1)],
                                attn_r[:, kt, 512 * chk:512 * (chk + 1)],
                                start=(kt == 0), stop=(kt == 1))
                    nc.vector.scalar_tensor_tensor(
                        x1u[:, mt, :], pp[:],
                        projb[:, mt:mt + 1],
                        x1[:, mt, :],
                        ALU.add, ALU.add)

                # ---------- phase 4: MLP ----------
                for mt in range(8):
                    pf = ps2a.tile([128, HW], F32, tag="mf",
                                   name=f"f1_{mt}")
                    for kt in range(2):
                        for chk in range(2):
                            nc.tensor.matmul(
                                pf[:, 512 * chk:512 * (chk + 1)],
                                fc1T[:, kt, 128 * mt:128 * (mt + 1)],
                                x1u[:, kt, 512 * chk:512 * (chk + 1)],
                                start=(kt == 0), stop=(kt == 1))
                    nc.scalar.activation(
                        h1[:, mt, :, 2:W + 2],
                        pf[:].rearrange("p (a b) -> p a b", a=H),
                        AF.Identity, bias=fc1b[:, mt:mt + 1])

            # depthwise 3x3: 9 diagonal taps; each tap's diagonal splits into
            # four independent [32,32] blocks at tile_position (32i,32i) that
            # run concurrently on the PE's sub-array grid.
            h1f = h1[:].rearrange("p c a b -> p c (a b)")
            # center + north taps run on the DVE (x-aligned fp16 ops +
            # one merge); the PE handles the 7 remaining taps
            taps = [(dy, dx) for dy in (-1, 0, 1) for dx in (-1, 0, 1)]
            taps.remove((0, 0))
            taps.remove((-1, 0))
            with tc.tile_pool(name="psd", bufs=2, space="PSUM") as psd, \
                 tc.tile_pool(name="ps2b", bufs=1, space="PSUM") as ps2b:
                for ct in range(8):
                    ps_dw = psd.tile([128, PADF], F32, tag="dw")
                    tc_t = pC.tile([128, H, W], FP16, tag="tct",
                                   name=f"tct_{ct % 2}")
                    nc.vector.tensor_scalar_mul(
                        tc_t[:], h1[:, ct, :, 2:W + 2], dwc[:, ct:ct + 1])
                    nc.vector.scalar_tensor_tensor(
                        tc_t[:, 1:H, :], h1[:, ct, 0:H - 1, 2:W + 2],
                        dwn[:, ct:ct + 1], tc_t[:, 1:H, :],
                        ALU.mult, ALU.add)
                    for ti, (dy, dx) in enumerate(taps):
                        shift = dy * WP + dx
                        chunks = _tap_chunks(shift)
                        for ci, (c0, n) in enumerate(chunks):
                            for qi in range(4):
                                nc.tensor.matmul(
                                    ps_dw[32 * qi:32 * qi + 32, c0:c0 + n],
                                    dwd[32 * qi:32 * qi + 32, ct,
                                        3 * (dy + 1) + (dx + 1),
                                        32 * qi:32 * qi + 32],
                                    h1f[32 * qi:32 * qi + 32, ct,
                                        c0 + shift:c0 + shift + n],
                                    start=(ti == 0),
                                    stop=(ti == len(taps) - 1
                                          and ci == len(chunks) - 1),
                                    tile_position=(32 * qi, 32 * qi))
                    ctr = ps_dw[:].rearrange("p (a b) -> p a b",
                                             a=H)[:, :, 2:W + 2]
                    nc.vector.tensor_add(ctr, tc_t[:], ctr)
                    nc.scalar.activation(hgr[:, ct, :], ctr, AF.Gelu)

                # fc2 + residual2
                for mt in range(2):
                    pg = ps2b.tile([128, 512], F32, tag="mm",
                                   name=f"f2_{mt}_0")
                    pg2 = ps2b.tile([128, 512], F32, tag="mm2",
                                    name=f"f2_{mt}_1")
                    pgs = [pg, pg2]
                    for kt in range(8):
                        for chk in range(2):
                            nc.tensor.matmul(
                                pgs[chk][:], fc2T[:, kt, 128 * mt:128 * (mt + 1)],
                                hgr[:, kt, 512 * chk:512 * (chk + 1)],
                                start=(kt == 0), stop=(kt == 7))
                    for chk in range(2):
                        nc.vector.scalar_tensor_tensor(
                            out_sb[:, mt, 512 * chk:512 * (chk + 1)], pgs[chk][:],
                            fc2b[:, mt:mt + 1],
                            x1u[:, mt, 512 * chk:512 * (chk + 1)],
                            ALU.add, ALU.add)
                    nc.sync.dma_start(outd[:, mt, :], out_sb[:, mt, :])


def _build_nc():
    if "nc" in _NC_CACHE:
        return _NC_CACHE["nc"]
    nc = bacc.Bacc(trn_type="TRN2", target_bir_lowering=False, debug=False)
    io = {}
    for name, shape, dt in [
        ("x1", [128, 2, HW], F32R), ("x2", [128, 2, HW], F32R),
        ("qT", [128, 2, C], F32R), ("kT", [128, 2, C], F32R),
        ("vwT", [128, 2, C], F32R), ("projT", [128, 2, C], F32R),
        ("fc1T", [128, 2, HID], F32R), ("fc2T", [128, 8, C], BF16),
        ("dwd", [128, 8, 9, 128], FP16),
        ("smalls", [128, 44], F32),
    ]:
        io[name] = nc.dram_tensor(name, shape, dt, kind="ExternalInput").ap()
    io["out"] = nc.dram_tensor("out", [128, 2, HW], F32, kind="ExternalOutput").ap()

    with tile.TileContext(nc) as tc:
        _build_body(nc, tc, io)
    nc.compile()
    _NC_CACHE["nc"] = nc
    return nc


def _to_part_layout(a, ntiles):
    """[ntiles*128, F] -> [128, ntiles, F] with c = kt*128 + p."""
    return np.ascontiguousarray(
        a.reshape(ntiles, 128, -1).transpose(1, 0, 2))


def _bias_layout(b, ntiles):
    """[ntiles*128] -> [128, ntiles]."""
    return np.ascontiguousarray(b.reshape(ntiles, 128).T)


def _prepare_weights(bn1_g, bn1_b, bn1_m, bn1_v, q_w, k_w, v_w, temp, proj_w,
                     proj_b, bn2_g, bn2_b, bn2_m, bn2_v, fc1_w, fc1_b, dw_w,
                     fc2_w, fc2_b):
    f64 = np.float64
    import ml_dtypes
    bf16 = ml_dtypes.bfloat16

    inv1 = (bn1_g.astype(f64) / np.sqrt(bn1_v.astype(f64) + EPS))
    beta1 = bn1_b.astype(f64) - bn1_m.astype(f64) * inv1
    inv2 = (bn2_g.astype(f64) / np.sqrt(bn2_v.astype(f64) + EPS))
    beta2 = bn2_b.astype(f64) - bn2_m.astype(f64) * inv2

    tscale = np.repeat(temp.astype(f64), DH)                     # [256]
    qw_f = q_w.astype(f64) * inv1[None, :] * tscale[:, None]
    qb = (q_w.astype(f64) @ beta1) * tscale
    kw_f = k_w.astype(f64) * inv1[None, :]
    kb = k_w.astype(f64) @ beta1
    fc1w_f = fc1_w.astype(f64) * inv2[None, :]
    fc1bf = fc1_b.astype(f64) + fc1_w.astype(f64) @ beta2

    # diag tap matrices, [k_row, ct, tap, m_col] so SBUF partition dim is k
    dwd = np.zeros((8, 9, 128, 128), np.float32)
    idx = np.arange(128)
    for ct in range(8):
        for t in range(9):
            dy, dx = t // 3, t % 3
            dwd[ct, t, idx, idx] = dw_w[ct * 128 + idx, 0, dy, dx]
    dwd = np.ascontiguousarray(dwd.transpose(2, 0, 1, 3))

    smalls = np.concatenate([
        _bias_layout(qb.astype(np.float32), 2),
        _bias_layout(kb.astype(np.float32), 2),
        _bias_layout(inv1.astype(np.float32), 2),
        _bias_layout(beta1.astype(np.float32), 2),
        _bias_layout(proj_b.astype(np.float32), 2),
        _bias_layout(fc2_b.astype(np.float32), 2),
        _bias_layout(fc1bf.astype(np.float32), 8),
        _bias_layout(dw_w[:, 0, 1, 1].astype(np.float32), 8),
        _bias_layout(dw_w[:, 0, 0, 1].astype(np.float32), 8),
        _bias_layout(dw_w[:, 0, 2, 1].astype(np.float32), 8),
    ], axis=1)
    w = {
        "qT": _to_part_layout(np.ascontiguousarray(qw_f.T).astype(np.float32), 2),
        "kT": _to_part_layout(np.ascontiguousarray(kw_f.T).astype(np.float32), 2),
        "vwT": _to_part_layout(np.ascontiguousarray(v_w.T).astype(np.float32), 2),
        "projT": _to_part_layout(np.ascontiguousarray(proj_w.T).astype(np.float32), 2),
        "fc1T": _to_part_layout(np.ascontiguousarray(fc1w_f.T).astype(np.float32), 2),
        "fc2T": _to_part_layout(np.ascontiguousarray(fc2_w.T).astype(np.float32), 8).astype(bf16),
        "dwd": dwd.astype(np.float16),
        "smalls": np.ascontiguousarray(smalls),
    }
    return w


_LAST_RESULTS = {}


def kernel(x1, x2, bn1_g, bn1_b, bn1_m, bn1_v, q_w, k_w, v_w, temp, proj_w,
           proj_b, bn2_g, bn2_b, bn2_m, bn2_v, fc1_w, fc1_b, dw_w, fc2_w,
           fc2_b, _trace=False):
    x1 = np.asarray(x1, np.float32)
    x2 = np.asarray(x2, np.float32)
    args = [np.asarray(a) for a in
            (bn1_g, bn1_b, bn1_m, bn1_v, q_w, k_w, v_w, temp, proj_w, proj_b,
             bn2_g, bn2_b, bn2_m, bn2_v, fc1_w, fc1_b, dw_w, fc2_w, fc2_b)]
    w = _prepare_weights(*args)

    nc = _build_nc()
    in_maps = []
    for i in range(N_CORES):
        m = dict(w)
        m["x1"] = _to_part_layout(x1[i].reshape(C, HW), 2)
        m["x2"] = _to_part_layout(x2[i].reshape(C, HW), 2)
        in_maps.append(m)

    res = run_bass_kernel_spmd(nc, in_maps, core_ids=list(range(N_CORES)),
                               trace=_trace)
    _LAST_RESULTS["res"] = res

    out = np.empty((B, C, H, W), np.float32)
    for i in range(N_CORES):
        o = res.results[i]["out"]                    # [128, 2, 1024]
        out[i] = o.transpose(1, 0, 2).reshape(C, H, W)
    return out
